# revision 59
# baseline (speedup 1.0000x reference)
"""MultiHeadAttention TRN2 Bass kernel (B=2, S=2048, D=1024, H=16, d=64).

Sharding: 8 cores = 2 (batch) x 4 (head groups of 4 heads).
Each core computes, for its batch b and head slice hs (256 dims):
    K^T = (Wk[hs,:] @ x_k^T + bk)    [256, 2048]   (dh on partitions)
    Q^T likewise; V = x_v @ Wv[hs,:].T + bv        [2048, 256]  (s on partitions)
    per head pair (2m, 2m+1): S^T = K_h @ Q_h^T, the two heads' score
    matmuls occupy disjoint PE row groups (contraction 64 at partitions
    0-63 / 64-127) and different PSUM banks -> they stream concurrently.
    P^T = exp(S^T / 8)   (scores ~ N(0,1), exp is safe without max-sub)
    [O^T ; denom] = [V_h | 1]^T @ P^T   (ones column folds the softmax
                                         denominator into the PV matmul)
    O^T = O^T * (1/denom)  (reciprocal_approx_fast, PE K=1 replicate)
    y_partial = O^T.T @ Wo[:, hs].T     [2048, 1024]
Host: y[b] = sum of 4 head-group partials + bo.

Everything the PE streams is bf16 (host-side cast: halves DMA, removes
all DVE casts, 1 cycle/row matmuls). The schedule is paced by the two
hard floors: Scalar-engine exp over 16.8M score elements (~137us) and
PE matmul rows (~110us). x is DMA'd in [128,512] column chunks through
rotating pools so the first score matmul lands ~10us in; after that the
emission keeps Scalar saturated: per sk-tile the PE emits scores(k)
BEFORE pv(k-1) (software pipeline, so the PE never blocks on the exp it
is feeding), and projection/output-projection work is woven into the
per-pair PE slack.
"""

import numpy as np
import ml_dtypes

import concourse.bass as bass
import concourse.tile as tile
import concourse.mybir as mybir
from concourse import bacc
from concourse.bass_utils import run_bass_kernel_spmd

D_MODEL = 1024
NUM_HEADS = 16
HEAD_DIM = 64
B, S = 2, 2048
N_CORES = 8
HG = 4                  # head-groups
HEADS_PER_CORE = NUM_HEADS // HG        # 4
DH = HEADS_PER_CORE * HEAD_DIM          # 256 output dims per core
KT = D_MODEL // 128                     # 8 contraction tiles
ST = S // 128                           # 16 sequence tiles
SB = S // 512                           # 4 sequence blocks of 512

F32 = mybir.dt.float32
F32R = mybir.dt.float32r
BF16 = mybir.dt.bfloat16
AF = mybir.ActivationFunctionType
BF16_NP = ml_dtypes.bfloat16
F8 = mybir.dt.float8e4
F8_NP = mybir.dt.np(F8)
DR = mybir.MatmulPerfMode.DoubleRow

_cached_nc = None


def build_nc():
    nc = bacc.Bacc("TRN2", target_bir_lowering=False, debug=False)

    xq_t = nc.declare_dram_parameter("xq_t", [128, KT * S], BF16, isOutput=False)
    xk_t = nc.declare_dram_parameter("xk_t", [128, KT * S], BF16, isOutput=False)
    xv_t = nc.declare_dram_parameter("xv_t", [128, KT * S], BF16, isOutput=False)
    wq_t = nc.declare_dram_parameter("wq_t", [128, KT * DH], BF16, isOutput=False)
    wk_t = nc.declare_dram_parameter("wk_t", [128, KT * DH], BF16, isOutput=False)
    wv_t = nc.declare_dram_parameter("wv_t", [128, KT * DH], BF16, isOutput=False)
    wo_t = nc.declare_dram_parameter("wo_t", [128, 2 * D_MODEL], BF16, isOutput=False)
    bqk = nc.declare_dram_parameter("bqk", [128, 4], F32, isOutput=False)
    bv = nc.declare_dram_parameter("bv", [1, DH], BF16, isOutput=False)
    y = nc.declare_dram_parameter("y", [S, D_MODEL], F32, isOutput=True)

    with tile.TileContext(nc) as tc:
        _emit(nc, tc, xq_t, xk_t, xv_t, wq_t, wk_t, wv_t, wo_t, bqk, bv, y)
    nc.compile()
    return nc


def _emit(nc, tc, xq_t, xk_t, xv_t, wq_t, wk_t, wv_t, wo_t, bqk, bv, y):
    from contextlib import ExitStack

    ctx = ExitStack()
    with ctx:
        # ---- persistent tiles -------------------------------------------
        persist = ctx.enter_context(tc.tile_pool(name="persist", bufs=1))
        qt = [persist.tile([128, S], BF16, tag=f"qt{m}", name=f"qt{m}")
              for m in range(2)]
        kt_sb = [persist.tile([128, S], BF16, tag=f"kt{m}", name=f"kt{m}")
                 for m in range(2)]
        v_sb = [persist.tile([128, HEADS_PER_CORE * 65], BF16, tag=f"v{i}",
                             name=f"v{i}") for i in range(ST)]
        ot = [persist.tile([128, S], BF16, tag=f"ot{m}", name=f"ot{m}")
              for m in range(2)]
        wo_flat = persist.tile([128, 2 * D_MODEL], BF16, tag="wof", name="wof")
        wo_r = [wo_flat[:, m * D_MODEL:(m + 1) * D_MODEL] for m in range(2)]
        ones_row = persist.tile([1, S], BF16, tag="ones")
        ones64 = persist.tile([33, 64], F32, tag="ones64")
        ones64_r = persist.tile([33, 64], F32R, tag="ones64r")
        ones_col = persist.tile([128, HEADS_PER_CORE], F32, tag="onesc")
        bqk_c = persist.tile([128, 4], F32, tag="bqk")  # bq|bk per-partition
        bq_c, bk_c = bqk_c[:, 0:2], bqk_c[:, 2:4]
        bv_r = persist.tile([1, DH], BF16, tag="bvr")
        w_flat = {n: persist.tile([128, KT * DH], BF16,
                                  tag=f"w{n}", name=f"w{n}")
                  for n in ("k", "q", "v")}
        w3 = {n: w_flat[n].rearrange("p (k d) -> p k d", d=DH)
              for n in ("k", "q", "v")}
        x_flat = {n: persist.tile([128, KT * S], BF16,
                                  tag=f"x{n}", name=f"x{n}")
                  for n in ("k", "q", "v")}
        x3 = {n: x_flat[n].rearrange("p (k s) -> p k s", s=S)
              for n in ("k", "q", "v")}

        # ---- DMA (priority order == consumption order) ------------------
        # fp8 x: per input 2MB; block-0 columns of every k-tile first so the
        # first projections can start ~10us in. Weights split across queues
        # (one transfer rides a single ~40GB/s DMA engine).
        def dma_cols(dst3, dram, name, c0, c1):
            # pair k-tiles per descriptor: SP dispatch is ~565ns per DMA,
            # and the lead is issue-rate bound
            dram3 = dram.rearrange("p (k s) -> p k s", s=S)
            for kp in range(KT // 2):
                nc.gpsimd.dma_start(
                    dst3[:, 2 * kp:2 * kp + 2, c0:c1],
                    dram3[:, 2 * kp:2 * kp + 2, c0:c1])

        def dma_w4(dst_tile, dram, n=4):
            w = dst_tile.shape[1] // n
            for j in range(n):
                nc.gpsimd.dma_start(dst_tile[:, j * w:(j + 1) * w],
                                    dram[:, j * w:(j + 1) * w])

        nc.gpsimd.dma_start(bqk_c[:], bqk[:, :])
        dma_w4(w_flat["k"], wk_t, 2)
        dma_cols(x3["k"], xk_t, "k", 0, 512)
        dma_w4(w_flat["q"], wq_t, 2)
        dma_cols(x3["q"], xq_t, "q", 0, 512)
        dma_cols(x3["k"], xk_t, "k", 512, S)
        dma_cols(x3["q"], xq_t, "q", 512, S)
        dma_w4(w_flat["v"], wv_t, 2)
        nc.gpsimd.dma_start(bv_r[:], bv[:])
        dma_cols(x3["v"], xv_t, "v", 0, S)
        dma_w4(wo_flat, wo_t)

        # ---- pipelined-body pools ---------------------------------------
        ps_s = ctx.enter_context(
            tc.tile_pool(name="pss", bufs=2, space="PSUM"))      # 4 banks
        ps_acc = ctx.enter_context(
            tc.tile_pool(name="psacc", bufs=1, space="PSUM"))    # 2 banks
        ps_w = ctx.enter_context(
            tc.tile_pool(name="psw", bufs=2, space="PSUM"))      # 2 banks
        pt_pool = ctx.enter_context(tc.tile_pool(name="pt", bufs=19))
        sm_pool = ctx.enter_context(tc.tile_pool(name="small", bufs=1))
        sm2_pool = ctx.enter_context(tc.tile_pool(name="small2", bufs=2))
        y_pool = ctx.enter_context(tc.tile_pool(name="ysb", bufs=2))

        # constants
        nc.gpsimd.memset(ones_row[:], 1.0)
        nc.vector.memset(ones64[:], 1.0)
        nc.vector.tensor_copy(ones64_r[:], ones64[:])
        nc.vector.memset(ones_col[:], 1.0)

        # ---- building blocks --------------------------------------------
        def proj_qk_m(name, dst, bias_c, nb, m):
            """Project one (512-col, m-half) block of Q^T or K^T (bf16)."""
            ps = ps_w.tile([128, 512], F32, tag="pw", name="pw")
            for k in range(KT):
                nc.tensor.matmul(
                    ps[:],
                    w3[name][:, k, m * 128:(m + 1) * 128],
                    x3[name][:, k, nb * 512:(nb + 1) * 512],
                    start=(k == 0), stop=(k == KT - 1),
                )
            nc.vector.tensor_scalar_add(
                dst[m][:, nb * 512:(nb + 1) * 512], ps[:],
                bias_c[:, m:m + 1])

        def v_chunk(i):
            """Project V for s-tile i into v_sb[i] (+ ones column). One
            accumulation group per PSUM tile: interleaved groups sharing a
            bank clobber each other's has_written state."""
            ps = ps_w.tile([128, 512], F32, tag="pw", name="pw")
            for k in range(KT):
                nc.tensor.matmul(
                    ps[:, 0:256],
                    x3["v"][:, k, i * 128:(i + 1) * 128],
                    w3["v"][:, k, :],
                    start=(k == 0), stop=False,
                )
            nc.tensor.matmul(
                ps[:, 0:256],
                ones_row[0:1, i * 128:(i + 1) * 128],
                bv_r[0:1, :],
                start=False, stop=True,
            )
            src = ps[:, 0:256].rearrange("p (h c) -> p h c", c=64)
            vv = v_sb[i].rearrange("p (h c) -> p h c", c=65)
            nc.vector.tensor_copy(vv[:, :, 0:64], src)
            nc.vector.tensor_copy(vv[:, :, 64], ones_col[:])

        def scores(qb, m, k):
            """Score pair (heads 2m,2m+1), sk-tile k, sq-block qb. The two
            K=64 matmuls use disjoint PE row groups + PSUM banks and stream
            concurrently. Returns the exp'd bf16 tile."""
            ss = ps_s.tile([128, 1024], F32, tag="ss", name="ss")
            for p2 in range(2):
                po = 64 * p2
                nc.tensor.matmul(
                    ss[:, p2 * 512:(p2 + 1) * 512],
                    kt_sb[m][po:po + 64, k * 128:(k + 1) * 128],
                    qt[m][po:po + 64, qb * 512:(qb + 1) * 512],
                    start=True, stop=True,
                )
            pt = pt_pool.tile([128, 1024], BF16, tag="pt", name="pt")
            nc.scalar.activation(
                pt[:], ss[:], AF.Exp, scale=1.0 / float(np.sqrt(HEAD_DIM)))
            return pt

        def pv(m, k, pt, accs):
            for p2 in range(2):
                h = 2 * m + p2
                nc.tensor.matmul(
                    accs[p2][:],
                    v_sb[k][:, h * 65:(h + 1) * 65],
                    pt[:, p2 * 512:(p2 + 1) * 512],
                    start=(k == 0), stop=(k == ST - 1),
                )

        def norm_stage1(accs):
            """Evict O rows + denominators to SBUF (frees the PSUM accs for
            the next pair immediately) and start the batched reciprocal."""
            o_sb = []
            den2 = sm_pool.tile([33, 512], F32, tag="den2", name="den2")
            for p2 in range(2):
                o = sm2_pool.tile([64, 512], BF16, tag=f"o{p2}", name="osb")
                nc.vector.tensor_copy(o[:], accs[p2][0:64, :])
                o_sb.append(o)
                nc.vector.tensor_copy(den2[32 * p2:32 * p2 + 1, :],
                                      accs[p2][64:65, :])
            recip2 = sm2_pool.tile([33, 512], F32R, tag="recip2", name="recip2")
            with nc.allow_low_precision(reason="softmax denom"):
                nc.vector.reciprocal(recip2[:], den2[:])
            return (o_sb, recip2)

        def norm_apply(qb, m, st):
            """ot[m][:, qb block] = O^T * recip: PE K=1 replicate + GpSimd
            multiply (SBUF-only operands, keeps DVE free)."""
            o_sb, recip2 = st
            for p2 in range(2):
                rep = ps_w.tile([128, 512], F32, tag="pw", name="pw")
                nc.tensor.matmul(
                    rep[0:64, :], ones64_r[32 * p2:32 * p2 + 1, :],
                    recip2[32 * p2:32 * p2 + 1, :],
                    start=True, stop=True,
                )
                rep_sb = sm_pool.tile([64, 512], BF16, tag="repsb",
                                      name="repsb")
                nc.vector.tensor_copy(rep_sb[:], rep[0:64, :])
                po = 64 * p2
                nc.gpsimd.tensor_mul(
                    ot[m][po:po + 64, qb * 512:(qb + 1) * 512],
                    o_sb[p2][:], rep_sb[:])

        def yproj_i(i, ysb_holder):
            """Output projection for s-tile i; DMA when both halves done."""
            if ysb_holder[0] is None:
                ysb_holder[0] = y_pool.tile([128, D_MODEL], F32, tag="ysb", name="ysb")
            ysb = ysb_holder[0]
            for nb2 in range(2):
                ps = ps_w.tile([128, 512], F32, tag="pw", name="pw")
                for m in range(2):
                    nc.tensor.matmul(
                        ps[:],
                        ot[m][:, i * 128:(i + 1) * 128],
                        wo_r[m][:, nb2 * 512:(nb2 + 1) * 512],
                        start=(m == 0), stop=(m == 1),
                    )
                nc.vector.tensor_copy(
                    ysb[:, nb2 * 512:(nb2 + 1) * 512], ps[:])
                nc.sync.dma_start(
                    y[i * 128:(i + 1) * 128, nb2 * 512:(nb2 + 1) * 512],
                    ysb[:, nb2 * 512:(nb2 + 1) * 512])
            ysb_holder[0] = None

        # =============== emission schedule ===============================
        # Window pipeline: window p emits scores of pair P_p while the
        # previous pair's PV drains at the same sk index (its exp finished a
        # whole window ago). Pair p's softmax norm is staged (SBUF evict +
        # batched reciprocal) at the end of window p+1 and applied early in
        # window p+2; projections and the output projection fill PE slack.
        pairs = [(qb, m) for qb in range(SB) for m in range(2)]
        yh = [None]

        def alloc_accs():
            return [ps_acc.tile([65, 512], F32, tag=f"acc{pp}",
                                name=f"acc{pp}") for pp in range(2)]

        proj_slots = {
            (0, 2): ("k", 0, 1), (0, 3): ("k", 1, 1),
            (0, 4): ("k", 0, 2), (0, 5): ("k", 1, 2),
            (0, 6): ("k", 0, 3), (0, 7): ("k", 1, 3),
            (0, 9): ("q", 0, 1), (0, 12): ("q", 1, 1),
            (2, 6): ("q", 0, 2), (2, 11): ("q", 1, 2),
            (4, 6): ("q", 0, 3), (4, 11): ("q", 1, 3),
        }
        yproj_slots = {
            (3, 6): 0, (3, 9): 1, (3, 12): 2, (4, 2): 3,     # yproj(0)
            (5, 6): 4, (5, 9): 5, (5, 12): 6, (6, 2): 7,     # yproj(1)
            (7, 6): 8, (7, 9): 9, (7, 12): 10, (7, 14): 11,  # yproj(2)
        }

        # lead-in: K block 0 + Q block 0, both m halves
        proj_qk_m("k", kt_sb, bk_c, 0, 0)
        proj_qk_m("k", kt_sb, bk_c, 0, 1)
        proj_qk_m("q", qt, bq_c, 0, 0)
        proj_qk_m("q", qt, bq_c, 0, 1)

        pts_prev = None
        accs_run = None
        apply_q = []            # FIFO of (qb, m, stage1 state)
        for p in range(len(pairs)):
            qb, m = pairs[p]
            if p >= 1:
                accs_run = alloc_accs()
            pts_cur = []
            for k in range(ST):
                pts_cur.append(scores(qb, m, k))
                if k == 5 and apply_q:
                    norm_apply(*apply_q.pop(0))
                if p == 1:
                    v_chunk(k)
                if p >= 1:
                    pv(pairs[p - 1][1], k, pts_prev[k], accs_run)
                if (p, k) in proj_slots:
                    nm, pm, pnb = proj_slots[(p, k)]
                    proj_qk_m(nm, kt_sb if nm == "k" else qt,
                              bk_c if nm == "k" else bq_c, pnb, pm)
                if (p, k) in yproj_slots:
                    yproj_i(yproj_slots[(p, k)], yh)
            if p >= 1:
                st = norm_stage1(accs_run)
                apply_q.append((pairs[p - 1][0], pairs[p - 1][1], st))
            pts_prev = pts_cur

        # tail window: PV + norm of the last pair, then yproj of block 3
        accs_run = alloc_accs()
        for k in range(ST):
            if k == 5 and apply_q:
                norm_apply(*apply_q.pop(0))
            pv(pairs[-1][1], k, pts_prev[k], accs_run)
        st = norm_stage1(accs_run)
        norm_apply(pairs[-1][0], pairs[-1][1], st)
        for i4 in range(4):
            yproj_i(3 * 4 + i4, yh)


def _get_nc():
    global _cached_nc
    if _cached_nc is None:
        _cached_nc = build_nc()
    return _cached_nc


def _make_in_maps(query, key, value, Wq, bq, Wk, bk, Wv, bv, Wo):
    """Shard + transpose + bf16-cast on host: core c = (b, hg), b = c // HG."""
    query = np.asarray(query, dtype=np.float32)
    key = np.asarray(key, dtype=np.float32)
    value = np.asarray(value, dtype=np.float32)
    Wq, Wk, Wv, Wo = (np.asarray(w, dtype=np.float32) for w in (Wq, Wk, Wv, Wo))
    bq, bk, bv = (np.asarray(b_, dtype=np.float32) for b_ in (bq, bk, bv))
    in_maps = []

    def tile_x(xt, dt):      # [1024, 2048] -> [128, 8*2048] k-tiled
        return np.ascontiguousarray(
            xt.reshape(KT, 128, S).transpose(1, 0, 2).reshape(128, KT * S)
        ).astype(dt)

    xq_t = [tile_x(query[b].T, BF16_NP) for b in range(B)]
    xk_t = [tile_x(key[b].T, BF16_NP) for b in range(B)]
    xv_t = [tile_x(value[b].T, BF16_NP) for b in range(B)]

    def tile_w(WT, dt=BF16_NP):  # [1024, 256] -> [128, 8*256] k-tiled
        return np.ascontiguousarray(
            WT.reshape(KT, 128, DH).transpose(1, 0, 2).reshape(128, KT * DH)
        ).astype(dt)

    for c in range(N_CORES):
        b, hg = divmod(c, HG)
        hs = slice(hg * DH, (hg + 1) * DH)
        wo_tiled = np.ascontiguousarray(
            Wo[:, hs].T.reshape(2, 128, D_MODEL).transpose(1, 0, 2)
            .reshape(128, 2 * D_MODEL)).astype(BF16_NP)
        bqk_pack = np.concatenate(
            [bq[hs].reshape(2, 128).T, bk[hs].reshape(2, 128).T],
            axis=1)          # [128, 4] = bq cols | bk cols
        in_maps.append({
            "xq_t": xq_t[b],
            "xk_t": xk_t[b],
            "xv_t": xv_t[b],
            "wq_t": tile_w(Wq[hs, :].T),
            "wk_t": tile_w(Wk[hs, :].T),
            "wv_t": tile_w(Wv[hs, :].T),
            "wo_t": wo_tiled,
            "bqk": np.ascontiguousarray(bqk_pack),
            "bv": np.ascontiguousarray(bv[hs]).reshape(1, DH).astype(BF16_NP),
        })
    return in_maps


def run(inputs, trace=False, **spmd_kwargs):
    nc = _get_nc()
    in_maps = _make_in_maps(
        inputs["query"], inputs["key"], inputs["value"],
        inputs["Wq"], inputs["bq"], inputs["Wk"], inputs["bk"],
        inputs["Wv"], inputs["bv"], inputs["Wo"])
    res = run_bass_kernel_spmd(
        nc, in_maps, list(range(N_CORES)), trace=trace, **spmd_kwargs)
    bo = np.asarray(inputs["bo"], dtype=np.float32)
    out = np.empty((B, S, D_MODEL), dtype=np.float32)
    for b in range(B):
        acc = np.zeros((S, D_MODEL), dtype=np.float32)
        for hg in range(HG):
            acc += res.results[b * HG + hg]["y"]
        out[b] = acc + bo
    return out, res


def kernel(**inputs) -> np.ndarray:
    out, _ = run(inputs, trace=False)
    return out


# revision 60
# speedup vs baseline: 1.0071x; 1.0071x over previous
"""MultiHeadAttention TRN2 Bass kernel (B=2, S=2048, D=1024, H=16, d=64).

Sharding: 8 cores = 2 (batch) x 4 (head groups of 4 heads).
Each core computes, for its batch b and head slice hs (256 dims):
    K^T = (Wk[hs,:] @ x_k^T + bk)    [256, 2048]   (dh on partitions)
    Q^T likewise; V = x_v @ Wv[hs,:].T + bv        [2048, 256]  (s on partitions)
    per head pair (2m, 2m+1): S^T = K_h @ Q_h^T, the two heads' score
    matmuls occupy disjoint PE row groups (contraction 64 at partitions
    0-63 / 64-127) and different PSUM banks -> they stream concurrently.
    P^T = exp(S^T / 8)   (scores ~ N(0,1), exp is safe without max-sub)
    [O^T ; denom] = [V_h | 1]^T @ P^T   (ones column folds the softmax
                                         denominator into the PV matmul)
    O^T = O^T * (1/denom)  (reciprocal_approx_fast, PE K=1 replicate)
    y_partial = O^T.T @ Wo[:, hs].T     [2048, 1024]
Host: y[b] = sum of 4 head-group partials + bo.

Everything the PE streams is bf16 (host-side cast: halves DMA, removes
all DVE casts, 1 cycle/row matmuls). The schedule is paced by the two
hard floors: Scalar-engine exp over 16.8M score elements (~137us) and
PE matmul rows (~110us). x is DMA'd in [128,512] column chunks through
rotating pools so the first score matmul lands ~10us in; after that the
emission keeps Scalar saturated: per sk-tile the PE emits scores(k)
BEFORE pv(k-1) (software pipeline, so the PE never blocks on the exp it
is feeding), and projection/output-projection work is woven into the
per-pair PE slack.
"""

import numpy as np
import ml_dtypes

import concourse.bass as bass
import concourse.tile as tile
import concourse.mybir as mybir
from concourse import bacc
from concourse.bass_utils import run_bass_kernel_spmd

D_MODEL = 1024
NUM_HEADS = 16
HEAD_DIM = 64
B, S = 2, 2048
N_CORES = 8
HG = 4                  # head-groups
HEADS_PER_CORE = NUM_HEADS // HG        # 4
DH = HEADS_PER_CORE * HEAD_DIM          # 256 output dims per core
KT = D_MODEL // 128                     # 8 contraction tiles
ST = S // 128                           # 16 sequence tiles
SB = S // 512                           # 4 sequence blocks of 512

F32 = mybir.dt.float32
F32R = mybir.dt.float32r
BF16 = mybir.dt.bfloat16
AF = mybir.ActivationFunctionType
BF16_NP = ml_dtypes.bfloat16
F8 = mybir.dt.float8e4
F8_NP = mybir.dt.np(F8)
DR = mybir.MatmulPerfMode.DoubleRow

_cached_nc = None


def build_nc():
    nc = bacc.Bacc("TRN2", target_bir_lowering=False, debug=False)

    xq_t = nc.declare_dram_parameter("xq_t", [128, KT * S], BF16, isOutput=False)
    xk_t = nc.declare_dram_parameter("xk_t", [128, KT * S], BF16, isOutput=False)
    xv_t = nc.declare_dram_parameter("xv_t", [128, KT * S], BF16, isOutput=False)
    wq_t = nc.declare_dram_parameter("wq_t", [128, KT * DH], BF16, isOutput=False)
    wk_t = nc.declare_dram_parameter("wk_t", [128, KT * DH], BF16, isOutput=False)
    wv_t = nc.declare_dram_parameter("wv_t", [128, KT * DH], BF16, isOutput=False)
    wo_t = nc.declare_dram_parameter("wo_t", [128, 2 * D_MODEL], BF16, isOutput=False)
    bqk = nc.declare_dram_parameter("bqk", [128, 4], F32, isOutput=False)
    bv = nc.declare_dram_parameter("bv", [1, DH], BF16, isOutput=False)
    y = nc.declare_dram_parameter("y", [S, D_MODEL], F32, isOutput=True)

    with tile.TileContext(nc) as tc:
        _emit(nc, tc, xq_t, xk_t, xv_t, wq_t, wk_t, wv_t, wo_t, bqk, bv, y)
    nc.compile()
    return nc


def _emit(nc, tc, xq_t, xk_t, xv_t, wq_t, wk_t, wv_t, wo_t, bqk, bv, y):
    from contextlib import ExitStack

    ctx = ExitStack()
    with ctx:
        # ---- persistent tiles -------------------------------------------
        persist = ctx.enter_context(tc.tile_pool(name="persist", bufs=1))
        qt = [persist.tile([128, S], BF16, tag=f"qt{m}", name=f"qt{m}")
              for m in range(2)]
        kt_sb = [persist.tile([128, S], BF16, tag=f"kt{m}", name=f"kt{m}")
                 for m in range(2)]
        v_sb = [persist.tile([128, HEADS_PER_CORE * 65], BF16, tag=f"v{i}",
                             name=f"v{i}") for i in range(ST)]
        ot = [persist.tile([128, S], BF16, tag=f"ot{m}", name=f"ot{m}")
              for m in range(2)]
        wo_flat = persist.tile([128, 2 * D_MODEL], BF16, tag="wof", name="wof")
        wo_r = [wo_flat[:, m * D_MODEL:(m + 1) * D_MODEL] for m in range(2)]
        ones_row = persist.tile([1, S], BF16, tag="ones")
        ones64 = persist.tile([33, 64], F32, tag="ones64")
        ones64_r = persist.tile([33, 64], F32R, tag="ones64r")
        ones_col = persist.tile([128, HEADS_PER_CORE], F32, tag="onesc")
        bqk_c = persist.tile([128, 4], F32, tag="bqk")  # bq|bk per-partition
        bq_c, bk_c = bqk_c[:, 0:2], bqk_c[:, 2:4]
        bv_r = persist.tile([1, DH], BF16, tag="bvr")
        w_flat = {n: persist.tile([128, KT * DH], BF16,
                                  tag=f"w{n}", name=f"w{n}")
                  for n in ("k", "q", "v")}
        w3 = {n: w_flat[n].rearrange("p (k d) -> p k d", d=DH)
              for n in ("k", "q", "v")}
        x_flat = {n: persist.tile([128, KT * S], BF16,
                                  tag=f"x{n}", name=f"x{n}")
                  for n in ("k", "q", "v")}
        x3 = {n: x_flat[n].rearrange("p (k s) -> p k s", s=S)
              for n in ("k", "q", "v")}

        # ---- DMA (priority order == consumption order) ------------------
        # fp8 x: per input 2MB; block-0 columns of every k-tile first so the
        # first projections can start ~10us in. Weights split across queues
        # (one transfer rides a single ~40GB/s DMA engine).
        def dma_cols(dst3, dram, name, c0, c1):
            # pair k-tiles per descriptor: SP dispatch is ~565ns per DMA,
            # and the lead is issue-rate bound
            dram3 = dram.rearrange("p (k s) -> p k s", s=S)
            for kp in range(KT // 2):
                nc.sync.dma_start(
                    dst3[:, 2 * kp:2 * kp + 2, c0:c1],
                    dram3[:, 2 * kp:2 * kp + 2, c0:c1])

        def dma_w4(dst_tile, dram, n=4):
            w = dst_tile.shape[1] // n
            for j in range(n):
                nc.sync.dma_start(dst_tile[:, j * w:(j + 1) * w],
                                    dram[:, j * w:(j + 1) * w])

        nc.sync.dma_start(bqk_c[:], bqk[:, :])
        dma_w4(w_flat["k"], wk_t, 2)
        dma_cols(x3["k"], xk_t, "k", 0, 512)
        dma_w4(w_flat["q"], wq_t, 2)
        dma_cols(x3["q"], xq_t, "q", 0, 512)
        dma_cols(x3["k"], xk_t, "k", 512, S)
        dma_cols(x3["q"], xq_t, "q", 512, S)
        dma_w4(w_flat["v"], wv_t, 2)
        nc.sync.dma_start(bv_r[:], bv[:])
        dma_cols(x3["v"], xv_t, "v", 0, S)
        dma_w4(wo_flat, wo_t)

        # ---- pipelined-body pools ---------------------------------------
        ps_s = ctx.enter_context(
            tc.tile_pool(name="pss", bufs=2, space="PSUM"))      # 4 banks
        ps_acc = ctx.enter_context(
            tc.tile_pool(name="psacc", bufs=1, space="PSUM"))    # 2 banks
        ps_w = ctx.enter_context(
            tc.tile_pool(name="psw", bufs=2, space="PSUM"))      # 2 banks
        pt_pool = ctx.enter_context(tc.tile_pool(name="pt", bufs=19))
        sm_pool = ctx.enter_context(tc.tile_pool(name="small", bufs=1))
        sm2_pool = ctx.enter_context(tc.tile_pool(name="small2", bufs=2))
        y_pool = ctx.enter_context(tc.tile_pool(name="ysb", bufs=2))

        # constants
        nc.gpsimd.memset(ones_row[:], 1.0)
        nc.vector.memset(ones64[:], 1.0)
        nc.vector.tensor_copy(ones64_r[:], ones64[:])
        nc.vector.memset(ones_col[:], 1.0)

        # ---- building blocks --------------------------------------------
        def proj_qk_m(name, dst, bias_c, nb, m):
            """Project one (512-col, m-half) block of Q^T or K^T (bf16)."""
            ps = ps_w.tile([128, 512], F32, tag="pw", name="pw")
            for k in range(KT):
                nc.tensor.matmul(
                    ps[:],
                    w3[name][:, k, m * 128:(m + 1) * 128],
                    x3[name][:, k, nb * 512:(nb + 1) * 512],
                    start=(k == 0), stop=(k == KT - 1),
                )
            nc.vector.tensor_scalar_add(
                dst[m][:, nb * 512:(nb + 1) * 512], ps[:],
                bias_c[:, m:m + 1])

        def v_chunk(i):
            """Project V for s-tile i into v_sb[i] (+ ones column). One
            accumulation group per PSUM tile: interleaved groups sharing a
            bank clobber each other's has_written state."""
            ps = ps_w.tile([128, 512], F32, tag="pw", name="pw")
            for k in range(KT):
                nc.tensor.matmul(
                    ps[:, 0:256],
                    x3["v"][:, k, i * 128:(i + 1) * 128],
                    w3["v"][:, k, :],
                    start=(k == 0), stop=False,
                )
            nc.tensor.matmul(
                ps[:, 0:256],
                ones_row[0:1, i * 128:(i + 1) * 128],
                bv_r[0:1, :],
                start=False, stop=True,
            )
            src = ps[:, 0:256].rearrange("p (h c) -> p h c", c=64)
            vv = v_sb[i].rearrange("p (h c) -> p h c", c=65)
            nc.vector.tensor_copy(vv[:, :, 0:64], src)
            nc.vector.tensor_copy(vv[:, :, 64], ones_col[:])

        def scores(qb, m, k):
            """Score pair (heads 2m,2m+1), sk-tile k, sq-block qb. The two
            K=64 matmuls use disjoint PE row groups + PSUM banks and stream
            concurrently. Returns the exp'd bf16 tile."""
            ss = ps_s.tile([128, 1024], F32, tag="ss", name="ss")
            for p2 in range(2):
                po = 64 * p2
                nc.tensor.matmul(
                    ss[:, p2 * 512:(p2 + 1) * 512],
                    kt_sb[m][po:po + 64, k * 128:(k + 1) * 128],
                    qt[m][po:po + 64, qb * 512:(qb + 1) * 512],
                    start=True, stop=True,
                )
            pt = pt_pool.tile([128, 1024], BF16, tag="pt", name="pt")
            nc.scalar.activation(
                pt[:], ss[:], AF.Exp, scale=1.0 / float(np.sqrt(HEAD_DIM)))
            return pt

        def pv(m, k, pt, accs):
            for p2 in range(2):
                h = 2 * m + p2
                nc.tensor.matmul(
                    accs[p2][:],
                    v_sb[k][:, h * 65:(h + 1) * 65],
                    pt[:, p2 * 512:(p2 + 1) * 512],
                    start=(k == 0), stop=(k == ST - 1),
                )

        def norm_stage1(accs):
            """Evict O rows + denominators to SBUF (frees the PSUM accs for
            the next pair immediately) and start the batched reciprocal."""
            o_sb = []
            den2 = sm_pool.tile([33, 512], F32, tag="den2", name="den2")
            for p2 in range(2):
                o = sm2_pool.tile([64, 512], BF16, tag=f"o{p2}", name="osb")
                nc.vector.tensor_copy(o[:], accs[p2][0:64, :])
                o_sb.append(o)
                nc.vector.tensor_copy(den2[32 * p2:32 * p2 + 1, :],
                                      accs[p2][64:65, :])
            recip2 = sm2_pool.tile([33, 512], F32R, tag="recip2", name="recip2")
            with nc.allow_low_precision(reason="softmax denom"):
                nc.vector.reciprocal(recip2[:], den2[:])
            return (o_sb, recip2)

        def norm_apply(qb, m, st):
            """ot[m][:, qb block] = O^T * recip: PE K=1 replicate + GpSimd
            multiply (SBUF-only operands, keeps DVE free)."""
            o_sb, recip2 = st
            for p2 in range(2):
                rep = ps_w.tile([128, 512], F32, tag="pw", name="pw")
                nc.tensor.matmul(
                    rep[0:64, :], ones64_r[32 * p2:32 * p2 + 1, :],
                    recip2[32 * p2:32 * p2 + 1, :],
                    start=True, stop=True,
                )
                rep_sb = sm_pool.tile([64, 512], BF16, tag="repsb",
                                      name="repsb")
                nc.vector.tensor_copy(rep_sb[:], rep[0:64, :])
                po = 64 * p2
                nc.gpsimd.tensor_mul(
                    ot[m][po:po + 64, qb * 512:(qb + 1) * 512],
                    o_sb[p2][:], rep_sb[:])

        def yproj_i(i, ysb_holder):
            """Output projection for s-tile i; DMA when both halves done."""
            if ysb_holder[0] is None:
                ysb_holder[0] = y_pool.tile([128, D_MODEL], F32, tag="ysb", name="ysb")
            ysb = ysb_holder[0]
            for nb2 in range(2):
                ps = ps_w.tile([128, 512], F32, tag="pw", name="pw")
                for m in range(2):
                    nc.tensor.matmul(
                        ps[:],
                        ot[m][:, i * 128:(i + 1) * 128],
                        wo_r[m][:, nb2 * 512:(nb2 + 1) * 512],
                        start=(m == 0), stop=(m == 1),
                    )
                nc.vector.tensor_copy(
                    ysb[:, nb2 * 512:(nb2 + 1) * 512], ps[:])
                nc.sync.dma_start(
                    y[i * 128:(i + 1) * 128, nb2 * 512:(nb2 + 1) * 512],
                    ysb[:, nb2 * 512:(nb2 + 1) * 512])
            ysb_holder[0] = None

        # =============== emission schedule ===============================
        # Window pipeline: window p emits scores of pair P_p while the
        # previous pair's PV drains at the same sk index (its exp finished a
        # whole window ago). Pair p's softmax norm is staged (SBUF evict +
        # batched reciprocal) at the end of window p+1 and applied early in
        # window p+2; projections and the output projection fill PE slack.
        pairs = [(qb, m) for qb in range(SB) for m in range(2)]
        yh = [None]

        def alloc_accs():
            return [ps_acc.tile([65, 512], F32, tag=f"acc{pp}",
                                name=f"acc{pp}") for pp in range(2)]

        proj_slots = {
            (0, 2): ("k", 0, 1), (0, 3): ("k", 1, 1),
            (0, 4): ("k", 0, 2), (0, 5): ("k", 1, 2),
            (0, 6): ("k", 0, 3), (0, 7): ("k", 1, 3),
            (0, 9): ("q", 0, 1), (0, 12): ("q", 1, 1),
            (2, 6): ("q", 0, 2), (2, 11): ("q", 1, 2),
            (4, 6): ("q", 0, 3), (4, 11): ("q", 1, 3),
        }
        yproj_slots = {
            (3, 6): 0, (3, 9): 1, (3, 12): 2, (4, 2): 3,     # yproj(0)
            (5, 6): 4, (5, 9): 5, (5, 12): 6, (6, 2): 7,     # yproj(1)
            (7, 6): 8, (7, 9): 9, (7, 12): 10, (7, 14): 11,  # yproj(2)
        }

        # lead-in: K block 0 + Q block 0, both m halves
        proj_qk_m("k", kt_sb, bk_c, 0, 0)
        proj_qk_m("k", kt_sb, bk_c, 0, 1)
        proj_qk_m("q", qt, bq_c, 0, 0)
        proj_qk_m("q", qt, bq_c, 0, 1)

        pts_prev = None
        accs_run = None
        apply_q = []            # FIFO of (qb, m, stage1 state)
        for p in range(len(pairs)):
            qb, m = pairs[p]
            if p >= 1:
                accs_run = alloc_accs()
            pts_cur = []
            for k in range(ST):
                pts_cur.append(scores(qb, m, k))
                if k == 5 and apply_q:
                    norm_apply(*apply_q.pop(0))
                if p == 1:
                    v_chunk(k)
                if p >= 1:
                    pv(pairs[p - 1][1], k, pts_prev[k], accs_run)
                if (p, k) in proj_slots:
                    nm, pm, pnb = proj_slots[(p, k)]
                    proj_qk_m(nm, kt_sb if nm == "k" else qt,
                              bk_c if nm == "k" else bq_c, pnb, pm)
                if (p, k) in yproj_slots:
                    yproj_i(yproj_slots[(p, k)], yh)
            if p >= 1:
                st = norm_stage1(accs_run)
                apply_q.append((pairs[p - 1][0], pairs[p - 1][1], st))
            pts_prev = pts_cur

        # tail window: PV + norm of the last pair, then yproj of block 3
        accs_run = alloc_accs()
        for k in range(ST):
            if k == 5 and apply_q:
                norm_apply(*apply_q.pop(0))
            pv(pairs[-1][1], k, pts_prev[k], accs_run)
        st = norm_stage1(accs_run)
        norm_apply(pairs[-1][0], pairs[-1][1], st)
        for i4 in range(4):
            yproj_i(3 * 4 + i4, yh)


def _get_nc():
    global _cached_nc
    if _cached_nc is None:
        _cached_nc = build_nc()
    return _cached_nc


def _make_in_maps(query, key, value, Wq, bq, Wk, bk, Wv, bv, Wo):
    """Shard + transpose + bf16-cast on host: core c = (b, hg), b = c // HG."""
    query = np.asarray(query, dtype=np.float32)
    key = np.asarray(key, dtype=np.float32)
    value = np.asarray(value, dtype=np.float32)
    Wq, Wk, Wv, Wo = (np.asarray(w, dtype=np.float32) for w in (Wq, Wk, Wv, Wo))
    bq, bk, bv = (np.asarray(b_, dtype=np.float32) for b_ in (bq, bk, bv))
    in_maps = []

    def tile_x(xt, dt):      # [1024, 2048] -> [128, 8*2048] k-tiled
        return np.ascontiguousarray(
            xt.reshape(KT, 128, S).transpose(1, 0, 2).reshape(128, KT * S)
        ).astype(dt)

    xq_t = [tile_x(query[b].T, BF16_NP) for b in range(B)]
    xk_t = [tile_x(key[b].T, BF16_NP) for b in range(B)]
    xv_t = [tile_x(value[b].T, BF16_NP) for b in range(B)]

    def tile_w(WT, dt=BF16_NP):  # [1024, 256] -> [128, 8*256] k-tiled
        return np.ascontiguousarray(
            WT.reshape(KT, 128, DH).transpose(1, 0, 2).reshape(128, KT * DH)
        ).astype(dt)

    for c in range(N_CORES):
        b, hg = divmod(c, HG)
        hs = slice(hg * DH, (hg + 1) * DH)
        wo_tiled = np.ascontiguousarray(
            Wo[:, hs].T.reshape(2, 128, D_MODEL).transpose(1, 0, 2)
            .reshape(128, 2 * D_MODEL)).astype(BF16_NP)
        bqk_pack = np.concatenate(
            [bq[hs].reshape(2, 128).T, bk[hs].reshape(2, 128).T],
            axis=1)          # [128, 4] = bq cols | bk cols
        in_maps.append({
            "xq_t": xq_t[b],
            "xk_t": xk_t[b],
            "xv_t": xv_t[b],
            "wq_t": tile_w(Wq[hs, :].T),
            "wk_t": tile_w(Wk[hs, :].T),
            "wv_t": tile_w(Wv[hs, :].T),
            "wo_t": wo_tiled,
            "bqk": np.ascontiguousarray(bqk_pack),
            "bv": np.ascontiguousarray(bv[hs]).reshape(1, DH).astype(BF16_NP),
        })
    return in_maps


def run(inputs, trace=False, **spmd_kwargs):
    nc = _get_nc()
    in_maps = _make_in_maps(
        inputs["query"], inputs["key"], inputs["value"],
        inputs["Wq"], inputs["bq"], inputs["Wk"], inputs["bk"],
        inputs["Wv"], inputs["bv"], inputs["Wo"])
    res = run_bass_kernel_spmd(
        nc, in_maps, list(range(N_CORES)), trace=trace, **spmd_kwargs)
    bo = np.asarray(inputs["bo"], dtype=np.float32)
    out = np.empty((B, S, D_MODEL), dtype=np.float32)
    for b in range(B):
        acc = np.zeros((S, D_MODEL), dtype=np.float32)
        for hg in range(HG):
            acc += res.results[b * HG + hg]["y"]
        out[b] = acc + bo
    return out, res


def kernel(**inputs) -> np.ndarray:
    out, _ = run(inputs, trace=False)
    return out
